# revision 15
# baseline (speedup 1.0000x reference)
"""Trainium2 Bass kernel for nn_AttentionLayer (B=4, S=2048, D=1024, H=16).

Self-contained: builds and compiles an SPMD Bass/Tile program once, then
runs it across 8 NeuronCores via run_bass_kernel_spmd.

Sharding (no collectives): core c handles batch b = c // 2 and query-token
half c % 2 (1024 query tokens). Each core receives pre-transposed fp8
activations (x^T slices) plus fp8/bf16 weights, computes its [1024, 1024]
slice of the final layernorm output in fp32, and the host reassembles.

v3 pipeline:
- K/Q^T/V projections and the FC context matmuls run in fp8 (e4m3) with
  DoubleRow perf mode: operands are staged as [128, 2, N] tiles holding
  two 128-row contraction slabs, halving matmul streaming time. Weights
  are host-prescaled by 64 (fp8 range); evacuations rescale by 1/64.
- Attention per head pair: scores^T = Kh @ Qh^T in bf16 with both heads'
  K=64 matmuls packed into one PSUM tile per query chunk — shared WAR
  deps make the scheduler emit them adjacently, so they row-tile onto
  disjoint PE-array halves and run concurrently.
- exp head A on ScalarE (native, bf16 out); head B on VectorE via a
  Schraudolph bit trick (one tensor_scalar fp32->int16 whose bits are
  bf16 exp values; the ~3% sawtooth error cancels in softmax).
- attn@V in bf16 with a per-head ones column producing denominators.
- Softmax normalization deferred: denominators roundtrip DRAM (bf16),
  reciprocal via a magic-number bit trick fused with the x64 fp8 scale,
  producing normalized fp8 context tiles in DoubleRow pair layout.
- FC: fp8 DR ctx matmuls + residual via identity matmuls (transposing
  Q^T tiles, identity prescaled by 4096 to match the fp8 scales) + bfc
  via a K=1 ones matmul; layernorm absorbs the 4096 scale (eps * 4096^2).
"""

import numpy as np
import ml_dtypes


from contextlib import ExitStack

import concourse.bass as bass
import concourse.tile as tile
import concourse.mybir as mybir
from concourse import bacc

F32 = mybir.dt.float32
BF16 = mybir.dt.bfloat16
I16 = mybir.dt.int16
I8 = mybir.dt.int8
F8 = mybir.dt.float8e4
DR = mybir.MatmulPerfMode.DoubleRow
AF = mybir.ActivationFunctionType
ALU = mybir.AluOpType

LOG2E = 1.4426950408889634
# exp(x/8) ~= bf16_bits(int16(x * SCH_A + SCH_B)) (Schraudolph, bf16 top bits)
SCH_A = float((1 << 23) * LOG2E) * 0.125 / 65536.0
SCH_B = (float(127 << 23) - 366393.0) / 65536.0
# same trick to fp8e4m3 bits directly (int8 out): exp(x/8) ~= f8_bits(i8)
SCH_A8 = SCH_A / 16.0
SCH_B8 = (SCH_B - 15360.0) / 16.0
# 1/x ~= bf16_bits(RCP_MAGIC - bf16_bits(x)), x > 0 (max rel err ~5%)
RCP_MAGIC = 0x7EF3
WSCALE = 64.0          # host fp8 weight prescale
LAM = WSCALE * WSCALE  # fc psum scale (ctx*64 @ Wfc*64); LN is scale-invariant


def bcast_ap(ap: bass.AP, parts: int) -> bass.AP:
    """Partition-broadcast a [1, N]-shaped DRAM AP to [parts, N]."""
    return bass.AP(tensor=ap.tensor, offset=ap.offset,
                   ap=[[0, parts]] + list(ap.ap[-1:]))


def nsplits(total, cap=512):
    return [(i, min(cap, total - i)) for i in range(0, total, cap)]


def build(T=1024, S=2048, D=1024, H=16, DK=64, n_cores=8, eps=1e-5,
          trn_type="TRN2", apply_affine=True, apply_bfc=True):
    assert DK == 64 and H % 2 == 0 and D == H * DK
    G = D // 256      # DoubleRow contraction groups (256 rows each)
    EB = D // 128     # e blocks (projection output chunks); == H//2
    TB = T // 128
    SB = S // 128
    PAIRS = H // 2
    VW = 65           # per-head vp stripe: 64 v columns + 1 ones column

    nc = bacc.Bacc(trn_type, target_bir_lowering=False, debug=False,
                   num_devices=n_cores)

    qT = nc.dram_tensor("qT", [D, T], BF16, kind="ExternalInput").ap()
    kT = nc.dram_tensor("kT", [D, S], F8, kind="ExternalInput").ap()
    vT = nc.dram_tensor("vT", [D, S], F8, kind="ExternalInput").ap()
    Wq = nc.dram_tensor("Wq", [D, D], BF16, kind="ExternalInput").ap()
    Wk = nc.dram_tensor("Wk", [D, D], F8, kind="ExternalInput").ap()
    Wv = nc.dram_tensor("Wv", [D, D], F8, kind="ExternalInput").ap()
    Wfc = nc.dram_tensor("Wfc", [D, D], F8, kind="ExternalInput").ap()
    bq = nc.dram_tensor("bq", [D], F32, kind="ExternalInput").ap()
    bk = nc.dram_tensor("bk", [D], F32, kind="ExternalInput").ap()
    bv = nc.dram_tensor("bv", [D], F32, kind="ExternalInput").ap()
    bfch = nc.dram_tensor("bfch", [D], BF16, kind="ExternalInput").ap()
    gamma = nc.dram_tensor("gamma", [D], F32, kind="ExternalInput").ap()
    beta = nc.dram_tensor("beta", [D], F32, kind="ExternalInput").ap()
    ident = nc.dram_tensor("ident", [128, 128], BF16, kind="ExternalInput").ap()
    out = nc.dram_tensor("out", [T, D], F32, kind="ExternalOutput").ap()

    den_dram = nc.dram_tensor("den_scratch", [H, T], BF16).ap()

    def load_dr(pool, src, n, tagp, chunk=None):
        """Load fp8 [D, n] DRAM tensor into G [128, 2, n] DoubleRow tiles."""
        tiles = [pool.tile([128, 2, n], F8, tag=f"{tagp}{g}", name=f"{tagp}{g}")
                 for g in range(G)]
        for c0, cn in nsplits(n, chunk or n):
            for g in range(G):
                t = tiles[g]
                nc.sync.dma_start(out=t[:, 0, c0:c0 + cn],
                                  in_=src[g * 256:g * 256 + 128, c0:c0 + cn])
                nc.sync.dma_start(out=t[:, 1, c0:c0 + cn],
                                  in_=src[g * 256 + 128:g * 256 + 256,
                                          c0:c0 + cn])
        return tiles

    with tile.TileContext(nc) as tc, ExitStack() as ctx:
        pconst = ctx.enter_context(tc.tile_pool(name="const", bufs=1))
        ppers = ctx.enter_context(tc.tile_pool(name="persist", bufs=1))

        # ---- tiny constants -------------------------------------------
        bqT = pconst.tile([128, EB], F32, tag="bqT", name="bqT")
        nc.sync.dma_start(out=bqT, in_=bq.rearrange("(e p) -> p e", p=128))
        bkT = pconst.tile([128, EB], F32, tag="bkT", name="bkT")
        nc.sync.dma_start(out=bkT, in_=bk.rearrange("(e p) -> p e", p=128))
        eps_t = pconst.tile([128, 1], F32, tag="eps", name="eps")
        nc.vector.memset(eps_t, eps * LAM * LAM)
        i_sb = pconst.tile([128, 128], BF16, tag="ident", name="ident")
        nc.sync.dma_start(out=i_sb, in_=ident)
        ones1 = pconst.tile([1, 128], BF16, tag="ones1", name="ones1")
        nc.vector.memset(ones1, 1.0)
        bfc_sb = pconst.tile([1, D], BF16, tag="bfc_sb", name="bfc_sb")
        nc.sync.dma_start(out=bfc_sb, in_=bcast_ap(bfch, 1))

        # ---- persistent tiles -----------------------------------------
        kpT8_sb = [ppers.tile([64, 2, S], F8, tag=f"kpT8_{e}",
                              name=f"kpT8_{e}") for e in range(EB)]
        qpT8_sb = [ppers.tile([64, 2, T], F8, tag=f"qpT8_{j}",
                              name=f"qpT8_{j}") for j in range(PAIRS)]
        vp_dr = [ppers.tile([128, 2, H * VW], F8, tag=f"vp{s2}",
                            name=f"vp{s2}") for s2 in range(SB // 2)]
        ctxT_sb = [ppers.tile([128, T], BF16, tag=f"ctxT{e}", name=f"ctxT{e}")
                   for e in range(EB)]
        ctx8_sb = [ppers.tile([128, 2, T], F8, tag=f"ctx8_{g}",
                              name=f"ctx8_{g}") for g in range(G)]
        qpT_sb = [ppers.tile([128, T], BF16, tag=f"qpT{j}", name=f"qpT{j}")
                  for j in range(PAIRS)]

        # ================= K projection (e-outer, fp8 DR) ==============
        with tc.tile_pool(name="wk", bufs=1) as pw, \
             tc.tile_pool(name="kx", bufs=1) as pkx, \
             tc.tile_pool(name="qx", bufs=1) as pqx, \
             tc.tile_pool(name="wq", bufs=1) as pwq, \
             tc.tile_pool(name="kst", bufs=2) as pkst, \
             tc.tile_pool(name="kps", bufs=2, space="PSUM") as pps, \
             tc.tile_pool(name="qps", bufs=2, space="PSUM") as pqps:
            wk_dr = load_dr(pw, Wk, D, "wk")
            kx_dr = load_dr(pkx, kT, S, "kx", chunk=1024)
            DB = D // 128
            qx_sb = [pqx.tile([128, T], BF16, tag=f"qx{d}", name=f"qx{d}")
                     for d in range(DB)]
            wq_sb = [pwq.tile([128, D], BF16, tag=f"wq{d}", name=f"wq{d}")
                     for d in range(DB)]
            for d in range(DB):
                nc.sync.dma_start(out=qx_sb[d], in_=qT[d * 128:(d + 1) * 128, :])
                nc.sync.dma_start(out=wq_sb[d], in_=Wq[d * 128:(d + 1) * 128, :])
            CK = min(S, 1024)
            for e in range(EB):
                ecol = slice(e * 128, (e + 1) * 128)
                for c0, cn in nsplits(S, CK):
                    ps = pps.tile([128, CK], F32, tag="kpT_ps", name="kpT_ps")
                    for g in range(G):
                        for n0, nn in nsplits(cn):
                            nc.tensor.matmul(
                                ps[:, n0:n0 + nn],
                                lhsT=wk_dr[g][:, :, ecol],
                                rhs=kx_dr[g][:, :, c0 + n0:c0 + n0 + nn],
                                start=(g == 0), stop=(g == G - 1),
                                perf_mode=DR)
                    kst = pkst.tile([128, CK], F8, tag="kst", name="kst")
                    nc.scalar.activation(
                        out=kst[:, 0:cn], in_=ps[:, 0:cn],
                        func=AF.Identity, scale=1.0 / WSCALE,
                        bias=bkT[:, e:e + 1])
                    kslc = kpT8_sb[e][:, :, c0:c0 + cn]
                    nc.sync.dma_start(out=kslc[0:32, 0, :],
                                      in_=kst[0:32, 0:cn])
                    nc.sync.dma_start(out=kslc[0:32, 1, :],
                                      in_=kst[32:64, 0:cn])
                    nc.sync.dma_start(out=kslc[32:64, 0, :],
                                      in_=kst[64:96, 0:cn])
                    nc.sync.dma_start(out=kslc[32:64, 1, :],
                                      in_=kst[96:128, 0:cn])

            # ============= Q^T projection (all pairs, fp8 DR) ==========
            for j in range(PAIRS):
                qps = pqps.tile([128, T], F32, tag="qps", name="qps")
                for d in range(DB):
                    for n0, nn in nsplits(T):
                        nc.tensor.matmul(
                            qps[:, n0:n0 + nn],
                            lhsT=wq_sb[d][:, j * 128:(j + 1) * 128],
                            rhs=qx_sb[d][:, n0:n0 + nn],
                            start=(d == 0), stop=(d == DB - 1))
                nc.scalar.activation(out=qpT_sb[j], in_=qps,
                                     func=AF.Identity, scale=1.0,
                                     bias=bqT[:, j:j + 1])
                qst = pkst.tile([128, T], F8, tag="qst", name="qst")
                nc.vector.tensor_scalar(out=qst, in0=qps,
                                        scalar1=bqT[:, j:j + 1],
                                        scalar2=None, op0=ALU.add)
                nc.sync.dma_start(out=qpT8_sb[j][0:32, 0, :],
                                  in_=qst[0:32, :])
                nc.sync.dma_start(out=qpT8_sb[j][0:32, 1, :],
                                  in_=qst[32:64, :])
                nc.sync.dma_start(out=qpT8_sb[j][32:64, 0, :],
                                  in_=qst[64:96, :])
                nc.sync.dma_start(out=qpT8_sb[j][32:64, 1, :],
                                  in_=qst[96:128, :])

        # ================= V projection (natural layout, fp8 DR) ========
        with tc.tile_pool(name="wv", bufs=1) as pw, \
             tc.tile_pool(name="vx", bufs=1) as pvx, \
             tc.tile_pool(name="vbc", bufs=1) as pvbc, \
             tc.tile_pool(name="vps", bufs=3, space="PSUM") as pps:
            bv_bc = pvbc.tile([128, D], F32, tag="bv_bc", name="bv_bc")
            nc.gpsimd.dma_start(out=bv_bc, in_=bcast_ap(bv, 128))
            wv_dr = load_dr(pw, Wv, D, "wv")
            vx_dr = load_dr(pvx, vT, S, "vx")
            for s in range(SB):
                ps = pps.tile([128, D], F32, tag="vp_ps", name="vp_ps")
                for g in range(G):
                    for n0, nn in nsplits(D):
                        nc.tensor.matmul(
                            ps[:, n0:n0 + nn],
                            lhsT=vx_dr[g][:, :, s * 128:(s + 1) * 128],
                            rhs=wv_dr[g][:, :, n0:n0 + nn],
                            start=(g == 0), stop=(g == G - 1), perf_mode=DR)
                vr = vp_dr[s // 2].rearrange("p k (h c) -> p k h c",
                                             c=VW)
                nc.vector.scalar_tensor_tensor(
                    out=vr[:, s % 2, :, 0:64],
                    in0=ps.rearrange("p (h c) -> p h c", c=DK),
                    scalar=1.0 / WSCALE,
                    in1=bv_bc.rearrange("p (h c) -> p h c", c=DK),
                    op0=ALU.mult, op1=ALU.add)
                nc.vector.memset(vr[:, s % 2, :, 64:65], 1.0)

        # ================= attention ====================================
        pwfc = ctx.enter_context(tc.tile_pool(name="wfc", bufs=1))
        wfc_dr = []
        for g in range(G):
            t = pwfc.tile([128, 2, D], F8, tag=f"wfc{g}", name=f"wfc{g}")
            nc.gpsimd.dma_start(out=t[:, 0, :],
                                in_=Wfc[g * 256:g * 256 + 128, :])
            nc.gpsimd.dma_start(out=t[:, 1, :],
                                in_=Wfc[g * 256 + 128:g * 256 + 256, :])
            wfc_dr.append(t)

        with tc.tile_pool(name="scp", bufs=3, space="PSUM") as psc, \
             tc.tile_pool(name="cxps", bufs=1, space="PSUM") as pcx, \
             tc.tile_pool(name="atA", bufs=2) as pata, \
             tc.tile_pool(name="atB", bufs=2) as patb, \
             tc.tile_pool(name="norm", bufs=2) as pnm, \
             tc.tile_pool(name="ctmp", bufs=2) as ptmp:
            HT = T // 2
            for j in range(PAIRS):
                kA = kpT8_sb[j][0:32, :, :]
                kB = kpT8_sb[j][32:64, :, :]
                for th in range(2):
                    tsl = slice(th * HT, (th + 1) * HT)
                    qA = qpT8_sb[j][0:32, :, tsl]
                    qB = qpT8_sb[j][32:64, :, tsl]
                    cxa = pcx.tile([VW, HT], F32, tag="cxA", name="cxA")
                    cxb = pcx.tile([VW, HT], F32, tag="cxB", name="cxB")
                    at_q = []

                    def attnv(m, cxa=cxa, cxb=cxb, j=j, at_q=at_q):
                        atA8, atB8 = at_q[m]
                        vrA = vp_dr[m][:, :, 2 * j * VW:2 * j * VW + VW]
                        vrB = vp_dr[m][:, :,
                                       (2 * j + 1) * VW:(2 * j + 2) * VW]
                        st, sp = (m == 0), (m == SB // 2 - 1)
                        nc.tensor.matmul(cxa, lhsT=vrA, rhs=atA8,
                                         start=st, stop=sp, perf_mode=DR)
                        nc.tensor.matmul(cxb, lhsT=vrB,
                                         rhs=atB8.bitcast(F8),
                                         start=st, stop=sp, perf_mode=DR)

                    for kb in range(SB):
                        kblk = slice(kb * 128, (kb + 1) * 128)
                        # one score tile holds both heads' chunk (A then
                        # B): shared WAR deps keep the two K=64 matmuls
                        # adjacent, so they row-tile concurrently; the
                        # 3-deep pool breaks the scores->exp->scores
                        # serial chain.
                        sc = psc.tile([128, T], F32, tag="sc", name="sc")
                        nc.tensor.matmul(sc[:, 0:HT],
                                         lhsT=kA[:, :, kblk], rhs=qA,
                                         start=True, stop=True,
                                         perf_mode=DR)
                        nc.tensor.matmul(sc[:, HT:T],
                                         lhsT=kB[:, :, kblk], rhs=qB,
                                         start=True, stop=True,
                                         perf_mode=DR)
                        # exp: head A on ScalarE (fp8 out), head B on
                        # VectorE (Schraudolph bits straight to fp8e4m3)
                        sl = kb % 2
                        if sl == 0:
                            atA8 = pata.tile([128, 2, HT], F8, tag="atA",
                                             name="atA")
                            atB8 = patb.tile([128, 2, HT], I8, tag="atB",
                                             name="atB")
                            at_q.append((atA8, atB8))
                        nc.scalar.activation(out=atA8[:, sl, :],
                                             in_=sc[:, 0:HT],
                                             func=AF.Exp, scale=0.125)
                        nc.vector.tensor_scalar(out=atB8[:, sl, :],
                                                in0=sc[:, HT:T],
                                                scalar1=SCH_A8,
                                                scalar2=SCH_B8,
                                                op0=ALU.mult, op1=ALU.add)
                        # attn@V (fp8 DoubleRow, 256-key contraction),
                        # lagged one kb-pair so its inputs are complete
                        # and it never stalls TensorE's in-order queue.
                        if sl == 1 and kb // 2 >= 1:
                            attnv(kb // 2 - 1)
                    attnv(SB // 2 - 1)
                    # evacuate ctx + denominators: head A via DVE, head B
                    # via ScalarE (the PSUM-capable engines); den rows
                    # ride along in the [65, HT] staging copies.
                    stga = ptmp.tile([VW, HT], BF16, tag="stga", name="stga")
                    nc.vector.tensor_copy(out=stga, in_=cxa)
                    stgb = ptmp.tile([VW, HT], BF16, tag="stgb", name="stgb")
                    nc.scalar.activation(out=stgb, in_=cxb, func=AF.Copy)
                    nc.sync.dma_start(out=ctxT_sb[j][0:64, tsl],
                                      in_=stga[0:64, :])
                    nc.sync.dma_start(out=ctxT_sb[j][64:128, tsl],
                                      in_=stgb[0:64, :])
                    nc.gpsimd.dma_start(out=den_dram[2 * j, tsl],
                                        in_=stga[64:65, :])
                    nc.gpsimd.dma_start(out=den_dram[2 * j + 1, tsl],
                                        in_=stgb[64:65, :])
                    # deferred softmax normalization: magic-number bf16
                    # reciprocal of broadcast denominators, the x64 fp8
                    # ctx scale folded into the magic constant.
                    dbc = pnm.tile([128, HT], BF16, tag="dbc", name="dbc")
                    nc.gpsimd.dma_start(
                        out=dbc[0:64, :],
                        in_=bcast_ap(den_dram[2 * j:2 * j + 1, tsl], 64))
                    nc.gpsimd.dma_start(
                        out=dbc[64:128, :],
                        in_=bcast_ap(den_dram[2 * j + 1:2 * j + 2, tsl], 64))
                    rbc = pnm.tile([128, HT], I16, tag="rbc", name="rbc")
                    nc.gpsimd.tensor_scalar(out=rbc, in0=dbc.bitcast(I16),
                                            scalar1=-1,
                                            scalar2=RCP_MAGIC + (6 << 7),
                                            op0=ALU.mult, op1=ALU.add)
                    nc.gpsimd.tensor_mul(out=ctx8_sb[j // 2][:, j % 2, tsl],
                                         in0=ctxT_sb[j][:, tsl],
                                         in1=rbc.bitcast(BF16))

        # ================= FC + residual + layernorm ====================
        with tc.tile_pool(name="fcps", bufs=2, space="PSUM") as pfc, \
             tc.tile_pool(name="lnbc", bufs=1) as plnb, \
             tc.tile_pool(name="xln", bufs=2) as px, \
             tc.tile_pool(name="stat", bufs=4) as pst:
            if apply_affine:
                gamma_bc = plnb.tile([128, D], F32, tag="gamma_bc",
                                     name="gamma_bc")
                nc.gpsimd.dma_start(out=gamma_bc, in_=bcast_ap(gamma, 128))
                beta_bc = plnb.tile([128, D], F32, tag="beta_bc",
                                    name="beta_bc")
                nc.gpsimd.dma_start(out=beta_bc, in_=bcast_ap(beta, 128))

            for t in range(TB):
                tblk = slice(t * 128, (t + 1) * 128)
                fc = pfc.tile([128, D], F32, tag="fc", name="fc")
                for c0, cn in nsplits(D):
                    for g in range(G):
                        nc.tensor.matmul(
                            fc[:, c0:c0 + cn],
                            lhsT=ctx8_sb[g][:, :, tblk],
                            rhs=wfc_dr[g][:, :, c0:c0 + cn],
                            start=(g == 0), stop=False, perf_mode=DR)
                    # residual: transpose qpT pair blocks via identity
                    # (identity prescaled by LAM to match fp8 scales)
                    for jj in range(c0 // 128, (c0 + cn) // 128):
                        nc.tensor.matmul(
                            fc[:, jj * 128:(jj + 1) * 128],
                            lhsT=qpT_sb[jj][:, tblk], rhs=i_sb,
                            start=False, stop=False)
                    if apply_bfc:
                        # bfc bias via K=1 ones matmul (marks group end)
                        nc.tensor.matmul(
                            fc[:, c0:c0 + cn], lhsT=ones1,
                            rhs=bfc_sb[0:1, c0:c0 + cn], start=False,
                            stop=True)
                ngr = max(D // 512, 1)
                gsz = min(D, 512)
                stats = pst.tile([128, ngr, 6], F32, tag="stats", name="stats")
                for g in range(ngr):
                    nc.vector.bn_stats(out=stats[:, g, :],
                                       in_=fc[:, g * gsz:(g + 1) * gsz])
                mv = pst.tile([128, 2], F32, tag="mv", name="mv")
                nc.vector.bn_aggr(out=mv, in_=stats)
                rstd = pst.tile([128, 1], F32, tag="rstd", name="rstd")
                nc.scalar.activation(out=rstd, in_=mv[:, 1:2], func=AF.Sqrt,
                                     bias=eps_t, scale=1.0)
                nc.vector.reciprocal(out=rstd, in_=rstd)
                nmr = pst.tile([128, 1], F32, tag="nmr", name="nmr")
                nc.vector.tensor_scalar(out=nmr, in0=mv[:, 0:1],
                                        scalar1=rstd, scalar2=-1.0,
                                        op0=ALU.mult, op1=ALU.mult)
                xn = px.tile([128, D], F32, tag="xn", name="xn")
                nc.scalar.activation(out=xn, in_=fc, func=AF.Identity,
                                     scale=rstd, bias=nmr)
                if apply_affine:
                    xg = px.tile([128, D], F32, tag="xg", name="xg")
                    nc.vector.tensor_mul(out=xg, in0=xn, in1=gamma_bc)
                    nc.gpsimd.tensor_add(out=xg, in0=xg, in1=beta_bc)
                else:
                    xg = xn
                nc.sync.dma_start(out=out[tblk, :], in_=xg)

    nc.compile()
    return nc


_B, _S, _D, _H, _DK = 4, 2048, 1024, 16, 64
_T = _S // 2
_NCORES = 8
_BF = ml_dtypes.bfloat16
_F8 = ml_dtypes.float8_e4m3

_nc_cache = {}


def _get_nc(apply_affine, apply_bfc):
    key = (apply_affine, apply_bfc)
    if key not in _nc_cache:
        _nc_cache[key] = build(T=_T, S=_S, D=_D, H=_H, DK=_DK,
                               n_cores=_NCORES, apply_affine=apply_affine,
                               apply_bfc=apply_bfc)
    return _nc_cache[key]


def _f8(x):
    return np.clip(x, -240.0, 240.0).astype(_F8)


def _execute(inputs, trace=False):
    from concourse.bass_utils import run_bass_kernel_spmd

    gamma_h = np.asarray(inputs["gamma"], np.float32)
    beta_h = np.asarray(inputs["beta"], np.float32)
    aff = not (np.all(gamma_h == 1.0) and np.all(beta_h == 0.0))
    bfc_h = np.asarray(inputs["bfc"], np.float32)
    nc = _get_nc(aff, bool(np.any(bfc_h != 0.0)))
    q = np.asarray(inputs["q"], np.float32)
    k = np.asarray(inputs["k"], np.float32)
    v = np.asarray(inputs["v"], np.float32)
    Wq = np.asarray(inputs["Wq"], np.float32).astype(_BF)
    Wk = _f8(np.asarray(inputs["Wk"], np.float32) * 64.0)
    Wv = _f8(np.asarray(inputs["Wv"], np.float32) * 64.0)
    Wfc = _f8(np.asarray(inputs["Wfc"], np.float32) * 64.0)
    fp = {n: np.asarray(inputs[n], np.float32)
          for n in ("bq", "bk", "bv", "gamma", "beta")}
    bfch = (np.asarray(inputs["bfc"], np.float32) * 4096.0).astype(_BF)
    ident = (np.eye(128, dtype=np.float32) * 4096.0).astype(_BF)

    in_maps = []
    for c in range(_NCORES):
        b, half = divmod(c, 2)
        t0 = half * _T
        in_maps.append({
            "qT": np.ascontiguousarray(q[b, t0:t0 + _T].T).astype(_BF),
            "kT": _f8(np.ascontiguousarray(k[b].T)),
            "vT": _f8(np.ascontiguousarray(v[b].T)),
            "Wq": Wq, "Wk": Wk, "Wv": Wv, "Wfc": Wfc,
            "bfch": bfch, "ident": ident, **fp,
        })

    res = run_bass_kernel_spmd(nc, in_maps, core_ids=list(range(_NCORES)),
                               trace=trace)
    out = np.empty((_B, _S, _D), np.float32)
    for c in range(_NCORES):
        b, half = divmod(c, 2)
        out[b, half * _T:(half + 1) * _T] = res.results[c]["out"]
    return out, res.exec_time_ns


def kernel(**inputs) -> np.ndarray:
    out, _ = _execute(inputs, trace=False)
    return out


# revision 16
# speedup vs baseline: 1.3948x; 1.3948x over previous
"""Trainium2 Bass kernel for nn_AttentionLayer (B=4, S=2048, D=1024, H=16).

Self-contained: builds and compiles an SPMD Bass/Tile program once, then
runs it across 8 NeuronCores via run_bass_kernel_spmd.

Sharding (no collectives): core c handles batch b = c // 2 and query-token
half c % 2 (1024 query tokens). Each core receives pre-transposed fp8
activations (x^T slices) plus fp8/bf16 weights, computes its [1024, 1024]
slice of the final layernorm output in fp32, and the host reassembles.

v3 pipeline:
- K/Q^T/V projections and the FC context matmuls run in fp8 (e4m3) with
  DoubleRow perf mode: operands are staged as [128, 2, N] tiles holding
  two 128-row contraction slabs, halving matmul streaming time. Weights
  are host-prescaled by 64 (fp8 range); evacuations rescale by 1/64.
- Attention per head pair: scores^T = Kh @ Qh^T in bf16 with both heads'
  K=64 matmuls packed into one PSUM tile per query chunk — shared WAR
  deps make the scheduler emit them adjacently, so they row-tile onto
  disjoint PE-array halves and run concurrently.
- exp head A on ScalarE (native, bf16 out); head B on VectorE via a
  Schraudolph bit trick (one tensor_scalar fp32->int16 whose bits are
  bf16 exp values; the ~3% sawtooth error cancels in softmax).
- attn@V in bf16 with a per-head ones column producing denominators.
- Softmax normalization deferred: denominators roundtrip DRAM (bf16),
  reciprocal via a magic-number bit trick fused with the x64 fp8 scale,
  producing normalized fp8 context tiles in DoubleRow pair layout.
- FC: fp8 DR ctx matmuls + residual via identity matmuls (transposing
  Q^T tiles, identity prescaled by 4096 to match the fp8 scales) + bfc
  via a K=1 ones matmul; layernorm absorbs the 4096 scale (eps * 4096^2).
"""

import numpy as np
import ml_dtypes


from contextlib import ExitStack

import concourse.bass as bass
import concourse.tile as tile
import concourse.mybir as mybir
from concourse import bacc

F32 = mybir.dt.float32
BF16 = mybir.dt.bfloat16
I16 = mybir.dt.int16
I8 = mybir.dt.int8
F8 = mybir.dt.float8e4
DR = mybir.MatmulPerfMode.DoubleRow
AF = mybir.ActivationFunctionType
ALU = mybir.AluOpType

LOG2E = 1.4426950408889634
# exp(x/8) ~= bf16_bits(int16(x * SCH_A + SCH_B)) (Schraudolph, bf16 top bits)
SCH_A = float((1 << 23) * LOG2E) * 0.125 / 65536.0
SCH_B = (float(127 << 23) - 366393.0) / 65536.0
# same trick to fp8e4m3 bits directly (int8 out): exp(x/8) ~= f8_bits(i8)
SCH_A8 = SCH_A / 16.0
SCH_B8 = (SCH_B - 15360.0) / 16.0
# 1/x ~= bf16_bits(RCP_MAGIC - bf16_bits(x)), x > 0 (max rel err ~5%)
RCP_MAGIC = 0x7EF3
WSCALE = 64.0          # host fp8 weight prescale
LAM = WSCALE * WSCALE  # fc psum scale (ctx*64 @ Wfc*64); LN is scale-invariant


def bcast_ap(ap: bass.AP, parts: int) -> bass.AP:
    """Partition-broadcast a [1, N]-shaped DRAM AP to [parts, N]."""
    return bass.AP(tensor=ap.tensor, offset=ap.offset,
                   ap=[[0, parts]] + list(ap.ap[-1:]))


def nsplits(total, cap=512):
    return [(i, min(cap, total - i)) for i in range(0, total, cap)]


def build(T=1024, S=2048, D=1024, H=16, DK=64, n_cores=8, eps=1e-5,
          trn_type="TRN2", apply_affine=True, apply_bfc=True):
    assert DK == 64 and H % 2 == 0 and D == H * DK
    G = D // 256      # DoubleRow contraction groups (256 rows each)
    EB = D // 128     # e blocks (projection output chunks); == H//2
    TB = T // 128
    SB = S // 128
    PAIRS = H // 2
    VW = 65           # per-head vp stripe: 64 v columns + 1 ones column

    nc = bacc.Bacc(trn_type, target_bir_lowering=False, debug=False,
                   num_devices=n_cores)

    qT = nc.dram_tensor("qT", [D, T], BF16, kind="ExternalInput").ap()
    kT = nc.dram_tensor("kT", [D, S], F8, kind="ExternalInput").ap()
    vT = nc.dram_tensor("vT", [D, S], F8, kind="ExternalInput").ap()
    Wq = nc.dram_tensor("Wq", [D, D], BF16, kind="ExternalInput").ap()
    Wk = nc.dram_tensor("Wk", [D, D], F8, kind="ExternalInput").ap()
    Wv = nc.dram_tensor("Wv", [D, D], F8, kind="ExternalInput").ap()
    Wfc = nc.dram_tensor("Wfc", [D, D], F8, kind="ExternalInput").ap()
    bq = nc.dram_tensor("bq", [D], F32, kind="ExternalInput").ap()
    bk = nc.dram_tensor("bk", [D], F32, kind="ExternalInput").ap()
    bv = nc.dram_tensor("bv", [D], F32, kind="ExternalInput").ap()
    bfch = nc.dram_tensor("bfch", [D], BF16, kind="ExternalInput").ap()
    gamma = nc.dram_tensor("gamma", [D], F32, kind="ExternalInput").ap()
    beta = nc.dram_tensor("beta", [D], F32, kind="ExternalInput").ap()
    ident = nc.dram_tensor("ident", [128, 128], BF16, kind="ExternalInput").ap()
    out = nc.dram_tensor("out", [T, D], F32, kind="ExternalOutput").ap()

    den_dram = nc.dram_tensor("den_scratch", [H, T], BF16).ap()

    def load_dr(pool, src, n, tagp, chunk=None):
        """Load fp8 [D, n] DRAM tensor into G [128, 2, n] DoubleRow tiles."""
        tiles = [pool.tile([128, 2, n], F8, tag=f"{tagp}{g}", name=f"{tagp}{g}")
                 for g in range(G)]
        for c0, cn in nsplits(n, chunk or n):
            for g in range(G):
                t = tiles[g]
                nc.sync.dma_start(out=t[:, 0, c0:c0 + cn],
                                  in_=src[g * 256:g * 256 + 128, c0:c0 + cn])
                nc.sync.dma_start(out=t[:, 1, c0:c0 + cn],
                                  in_=src[g * 256 + 128:g * 256 + 256,
                                          c0:c0 + cn])
        return tiles

    with tile.TileContext(nc) as tc, ExitStack() as ctx:
        pconst = ctx.enter_context(tc.tile_pool(name="const", bufs=1))
        ppers = ctx.enter_context(tc.tile_pool(name="persist", bufs=1))

        # ---- tiny constants -------------------------------------------
        bqT = pconst.tile([128, EB], F32, tag="bqT", name="bqT")
        nc.sync.dma_start(out=bqT, in_=bq.rearrange("(e p) -> p e", p=128))
        bkT = pconst.tile([128, EB], F32, tag="bkT", name="bkT")
        nc.sync.dma_start(out=bkT, in_=bk.rearrange("(e p) -> p e", p=128))
        eps_t = pconst.tile([128, 1], F32, tag="eps", name="eps")
        nc.vector.memset(eps_t, eps * LAM * LAM)
        i_sb = pconst.tile([128, 128], BF16, tag="ident", name="ident")
        nc.sync.dma_start(out=i_sb, in_=ident)
        ones1 = pconst.tile([1, 128], BF16, tag="ones1", name="ones1")
        nc.vector.memset(ones1, 1.0)
        bfc_sb = pconst.tile([1, D], BF16, tag="bfc_sb", name="bfc_sb")
        nc.sync.dma_start(out=bfc_sb, in_=bcast_ap(bfch, 1))

        # ---- persistent tiles -----------------------------------------
        kpT_sb = [ppers.tile([128, S], BF16, tag=f"kpT{e}", name=f"kpT{e}")
                  for e in range(EB)]
        vp_dr = [ppers.tile([128, 2, H * VW], F8, tag=f"vp{s2}",
                            name=f"vp{s2}") for s2 in range(SB // 2)]
        ctxT_sb = [ppers.tile([128, T], BF16, tag=f"ctxT{e}", name=f"ctxT{e}")
                   for e in range(EB)]
        ctx8_sb = [ppers.tile([128, 2, T], F8, tag=f"ctx8_{g}",
                              name=f"ctx8_{g}") for g in range(G)]
        qpT_sb = [ppers.tile([128, T], BF16, tag=f"qpT{j}", name=f"qpT{j}")
                  for j in range(PAIRS)]

        # ================= K projection (e-outer, fp8 DR) ==============
        with tc.tile_pool(name="wk", bufs=1) as pw, \
             tc.tile_pool(name="kx", bufs=1) as pkx, \
             tc.tile_pool(name="qx", bufs=1) as pqx, \
             tc.tile_pool(name="wq", bufs=1) as pwq, \
             tc.tile_pool(name="kps", bufs=2, space="PSUM") as pps, \
             tc.tile_pool(name="qps", bufs=2, space="PSUM") as pqps:
            wk_dr = load_dr(pw, Wk, D, "wk")
            kx_dr = load_dr(pkx, kT, S, "kx", chunk=1024)
            DB = D // 128
            qx_sb = [pqx.tile([128, T], BF16, tag=f"qx{d}", name=f"qx{d}")
                     for d in range(DB)]
            wq_sb = [pwq.tile([128, D], BF16, tag=f"wq{d}", name=f"wq{d}")
                     for d in range(DB)]
            for d in range(DB):
                nc.sync.dma_start(out=qx_sb[d], in_=qT[d * 128:(d + 1) * 128, :])
                nc.sync.dma_start(out=wq_sb[d], in_=Wq[d * 128:(d + 1) * 128, :])
            CK = min(S, 1024)
            for e in range(EB):
                ecol = slice(e * 128, (e + 1) * 128)
                for c0, cn in nsplits(S, CK):
                    ps = pps.tile([128, CK], F32, tag="kpT_ps", name="kpT_ps")
                    for g in range(G):
                        for n0, nn in nsplits(cn):
                            nc.tensor.matmul(
                                ps[:, n0:n0 + nn],
                                lhsT=wk_dr[g][:, :, ecol],
                                rhs=kx_dr[g][:, :, c0 + n0:c0 + n0 + nn],
                                start=(g == 0), stop=(g == G - 1),
                                perf_mode=DR)
                    nc.scalar.activation(
                        out=kpT_sb[e][:, c0:c0 + cn], in_=ps[:, 0:cn],
                        func=AF.Identity, scale=1.0 / WSCALE,
                        bias=bkT[:, e:e + 1])

            # ============= Q^T projection (all pairs, fp8 DR) ==========
            for j in range(PAIRS):
                qps = pqps.tile([128, T], F32, tag="qps", name="qps")
                for d in range(DB):
                    for n0, nn in nsplits(T):
                        nc.tensor.matmul(
                            qps[:, n0:n0 + nn],
                            lhsT=wq_sb[d][:, j * 128:(j + 1) * 128],
                            rhs=qx_sb[d][:, n0:n0 + nn],
                            start=(d == 0), stop=(d == DB - 1))
                nc.scalar.activation(out=qpT_sb[j], in_=qps,
                                     func=AF.Identity, scale=1.0,
                                     bias=bqT[:, j:j + 1])

        # ================= V projection (natural layout, fp8 DR) ========
        with tc.tile_pool(name="wv", bufs=1) as pw, \
             tc.tile_pool(name="vx", bufs=1) as pvx, \
             tc.tile_pool(name="vbc", bufs=1) as pvbc, \
             tc.tile_pool(name="vps", bufs=3, space="PSUM") as pps:
            bv_bc = pvbc.tile([128, D], F32, tag="bv_bc", name="bv_bc")
            nc.gpsimd.dma_start(out=bv_bc, in_=bcast_ap(bv, 128))
            wv_dr = load_dr(pw, Wv, D, "wv")
            vx_dr = load_dr(pvx, vT, S, "vx")
            for s in range(SB):
                ps = pps.tile([128, D], F32, tag="vp_ps", name="vp_ps")
                for g in range(G):
                    for n0, nn in nsplits(D):
                        nc.tensor.matmul(
                            ps[:, n0:n0 + nn],
                            lhsT=vx_dr[g][:, :, s * 128:(s + 1) * 128],
                            rhs=wv_dr[g][:, :, n0:n0 + nn],
                            start=(g == 0), stop=(g == G - 1), perf_mode=DR)
                vr = vp_dr[s // 2].rearrange("p k (h c) -> p k h c",
                                             c=VW)
                nc.vector.scalar_tensor_tensor(
                    out=vr[:, s % 2, :, 0:64],
                    in0=ps.rearrange("p (h c) -> p h c", c=DK),
                    scalar=1.0 / WSCALE,
                    in1=bv_bc.rearrange("p (h c) -> p h c", c=DK),
                    op0=ALU.mult, op1=ALU.add)
                nc.vector.memset(vr[:, s % 2, :, 64:65], 1.0)

        # ================= attention ====================================
        pwfc = ctx.enter_context(tc.tile_pool(name="wfc", bufs=1))
        wfc_dr = []
        for g in range(G):
            t = pwfc.tile([128, 2, D], F8, tag=f"wfc{g}", name=f"wfc{g}")
            nc.gpsimd.dma_start(out=t[:, 0, :],
                                in_=Wfc[g * 256:g * 256 + 128, :])
            nc.gpsimd.dma_start(out=t[:, 1, :],
                                in_=Wfc[g * 256 + 128:g * 256 + 256, :])
            wfc_dr.append(t)

        with tc.tile_pool(name="scp", bufs=3, space="PSUM") as psc, \
             tc.tile_pool(name="cxps", bufs=1, space="PSUM") as pcx, \
             tc.tile_pool(name="atA", bufs=2) as pata, \
             tc.tile_pool(name="atB", bufs=2) as patb, \
             tc.tile_pool(name="norm", bufs=2) as pnm, \
             tc.tile_pool(name="ctmp", bufs=2) as ptmp:
            HT = T // 2
            for j in range(PAIRS):
                kA = kpT_sb[j][0:64, :]
                kB = kpT_sb[j][64:128, :]
                for th in range(2):
                    tsl = slice(th * HT, (th + 1) * HT)
                    qA = qpT_sb[j][0:64, tsl]
                    qB = qpT_sb[j][64:128, tsl]
                    cxa = pcx.tile([VW, HT], F32, tag="cxA", name="cxA")
                    cxb = pcx.tile([VW, HT], F32, tag="cxB", name="cxB")
                    at_q = []

                    def attnv(m, cxa=cxa, cxb=cxb, j=j, at_q=at_q):
                        atA8, atB8 = at_q[m]
                        vrA = vp_dr[m][:, :, 2 * j * VW:2 * j * VW + VW]
                        vrB = vp_dr[m][:, :,
                                       (2 * j + 1) * VW:(2 * j + 2) * VW]
                        st, sp = (m == 0), (m == SB // 2 - 1)
                        nc.tensor.matmul(cxa, lhsT=vrA, rhs=atA8,
                                         start=st, stop=sp, perf_mode=DR)
                        nc.tensor.matmul(cxb, lhsT=vrB,
                                         rhs=atB8.bitcast(F8),
                                         start=st, stop=sp, perf_mode=DR)

                    for kb in range(SB):
                        kblk = slice(kb * 128, (kb + 1) * 128)
                        # one score tile holds both heads' chunk (A then
                        # B): shared WAR deps keep the two K=64 matmuls
                        # adjacent, so they row-tile concurrently; the
                        # 3-deep pool breaks the scores->exp->scores
                        # serial chain.
                        sc = psc.tile([128, T], F32, tag="sc", name="sc")
                        nc.tensor.matmul(sc[:, 0:HT], lhsT=kA[:, kblk],
                                         rhs=qA, start=True, stop=True)
                        nc.tensor.matmul(sc[:, HT:T], lhsT=kB[:, kblk],
                                         rhs=qB, start=True, stop=True)
                        # exp: head A on ScalarE (fp8 out), head B on
                        # VectorE (Schraudolph bits straight to fp8e4m3)
                        sl = kb % 2
                        if sl == 0:
                            atA8 = pata.tile([128, 2, HT], F8, tag="atA",
                                             name="atA")
                            atB8 = patb.tile([128, 2, HT], I8, tag="atB",
                                             name="atB")
                            at_q.append((atA8, atB8))
                        nc.scalar.activation(out=atA8[:, sl, :],
                                             in_=sc[:, 0:HT],
                                             func=AF.Exp, scale=0.125)
                        nc.vector.tensor_scalar(out=atB8[:, sl, :],
                                                in0=sc[:, HT:T],
                                                scalar1=SCH_A8,
                                                scalar2=SCH_B8,
                                                op0=ALU.mult, op1=ALU.add)
                        # attn@V (fp8 DoubleRow, 256-key contraction),
                        # lagged one kb-pair so its inputs are complete
                        # and it never stalls TensorE's in-order queue.
                        if sl == 1 and kb // 2 >= 1:
                            attnv(kb // 2 - 1)
                    attnv(SB // 2 - 1)
                    # evacuate ctx + denominators: head A via DVE, head B
                    # via ScalarE (the PSUM-capable engines); den rows
                    # ride along in the [65, HT] staging copies.
                    stga = ptmp.tile([VW, HT], BF16, tag="stga", name="stga")
                    nc.vector.tensor_copy(out=stga, in_=cxa)
                    stgb = ptmp.tile([VW, HT], BF16, tag="stgb", name="stgb")
                    nc.scalar.activation(out=stgb, in_=cxb, func=AF.Copy)
                    nc.sync.dma_start(out=ctxT_sb[j][0:64, tsl],
                                      in_=stga[0:64, :])
                    nc.sync.dma_start(out=ctxT_sb[j][64:128, tsl],
                                      in_=stgb[0:64, :])
                    nc.gpsimd.dma_start(out=den_dram[2 * j, tsl],
                                        in_=stga[64:65, :])
                    nc.gpsimd.dma_start(out=den_dram[2 * j + 1, tsl],
                                        in_=stgb[64:65, :])
                    # deferred softmax normalization: magic-number bf16
                    # reciprocal of broadcast denominators, the x64 fp8
                    # ctx scale folded into the magic constant.
                    dbc = pnm.tile([128, HT], BF16, tag="dbc", name="dbc")
                    nc.gpsimd.dma_start(
                        out=dbc[0:64, :],
                        in_=bcast_ap(den_dram[2 * j:2 * j + 1, tsl], 64))
                    nc.gpsimd.dma_start(
                        out=dbc[64:128, :],
                        in_=bcast_ap(den_dram[2 * j + 1:2 * j + 2, tsl], 64))
                    rbc = pnm.tile([128, HT], I16, tag="rbc", name="rbc")
                    nc.gpsimd.tensor_scalar(out=rbc, in0=dbc.bitcast(I16),
                                            scalar1=-1,
                                            scalar2=RCP_MAGIC + (6 << 7),
                                            op0=ALU.mult, op1=ALU.add)
                    nc.gpsimd.tensor_mul(out=ctx8_sb[j // 2][:, j % 2, tsl],
                                         in0=ctxT_sb[j][:, tsl],
                                         in1=rbc.bitcast(BF16))

        # ================= FC + residual + layernorm ====================
        with tc.tile_pool(name="fcps", bufs=2, space="PSUM") as pfc, \
             tc.tile_pool(name="lnbc", bufs=1) as plnb, \
             tc.tile_pool(name="xln", bufs=2) as px, \
             tc.tile_pool(name="stat", bufs=4) as pst:
            if apply_affine:
                gamma_bc = plnb.tile([128, D], F32, tag="gamma_bc",
                                     name="gamma_bc")
                nc.gpsimd.dma_start(out=gamma_bc, in_=bcast_ap(gamma, 128))
                beta_bc = plnb.tile([128, D], F32, tag="beta_bc",
                                    name="beta_bc")
                nc.gpsimd.dma_start(out=beta_bc, in_=bcast_ap(beta, 128))

            for t in range(TB):
                tblk = slice(t * 128, (t + 1) * 128)
                fc = pfc.tile([128, D], F32, tag="fc", name="fc")
                for c0, cn in nsplits(D):
                    for g in range(G):
                        nc.tensor.matmul(
                            fc[:, c0:c0 + cn],
                            lhsT=ctx8_sb[g][:, :, tblk],
                            rhs=wfc_dr[g][:, :, c0:c0 + cn],
                            start=(g == 0), stop=False, perf_mode=DR)
                    # residual: transpose qpT pair blocks via identity
                    # (identity prescaled by LAM to match fp8 scales)
                    for jj in range(c0 // 128, (c0 + cn) // 128):
                        nc.tensor.matmul(
                            fc[:, jj * 128:(jj + 1) * 128],
                            lhsT=qpT_sb[jj][:, tblk], rhs=i_sb,
                            start=False, stop=False)
                    if apply_bfc:
                        # bfc bias via K=1 ones matmul (marks group end)
                        nc.tensor.matmul(
                            fc[:, c0:c0 + cn], lhsT=ones1,
                            rhs=bfc_sb[0:1, c0:c0 + cn], start=False,
                            stop=True)
                ngr = max(D // 512, 1)
                gsz = min(D, 512)
                stats = pst.tile([128, ngr, 6], F32, tag="stats", name="stats")
                for g in range(ngr):
                    nc.vector.bn_stats(out=stats[:, g, :],
                                       in_=fc[:, g * gsz:(g + 1) * gsz])
                mv = pst.tile([128, 2], F32, tag="mv", name="mv")
                nc.vector.bn_aggr(out=mv, in_=stats)
                rstd = pst.tile([128, 1], F32, tag="rstd", name="rstd")
                nc.scalar.activation(out=rstd, in_=mv[:, 1:2], func=AF.Sqrt,
                                     bias=eps_t, scale=1.0)
                nc.vector.reciprocal(out=rstd, in_=rstd)
                nmr = pst.tile([128, 1], F32, tag="nmr", name="nmr")
                nc.vector.tensor_scalar(out=nmr, in0=mv[:, 0:1],
                                        scalar1=rstd, scalar2=-1.0,
                                        op0=ALU.mult, op1=ALU.mult)
                xn = px.tile([128, D], F32, tag="xn", name="xn")
                nc.scalar.activation(out=xn, in_=fc, func=AF.Identity,
                                     scale=rstd, bias=nmr)
                if apply_affine:
                    xg = px.tile([128, D], F32, tag="xg", name="xg")
                    nc.vector.tensor_mul(out=xg, in0=xn, in1=gamma_bc)
                    nc.gpsimd.tensor_add(out=xg, in0=xg, in1=beta_bc)
                else:
                    xg = xn
                nc.sync.dma_start(out=out[tblk, :], in_=xg)

    nc.compile()
    return nc


_B, _S, _D, _H, _DK = 4, 2048, 1024, 16, 64
_T = _S // 2
_NCORES = 8
_BF = ml_dtypes.bfloat16
_F8 = ml_dtypes.float8_e4m3

_nc_cache = {}


def _get_nc(apply_affine, apply_bfc):
    key = (apply_affine, apply_bfc)
    if key not in _nc_cache:
        _nc_cache[key] = build(T=_T, S=_S, D=_D, H=_H, DK=_DK,
                               n_cores=_NCORES, apply_affine=apply_affine,
                               apply_bfc=apply_bfc)
    return _nc_cache[key]


def _f8(x):
    return np.clip(x, -240.0, 240.0).astype(_F8)


def _execute(inputs, trace=False):
    from concourse.bass_utils import run_bass_kernel_spmd

    gamma_h = np.asarray(inputs["gamma"], np.float32)
    beta_h = np.asarray(inputs["beta"], np.float32)
    aff = not (np.all(gamma_h == 1.0) and np.all(beta_h == 0.0))
    bfc_h = np.asarray(inputs["bfc"], np.float32)
    nc = _get_nc(aff, bool(np.any(bfc_h != 0.0)))
    q = np.asarray(inputs["q"], np.float32)
    k = np.asarray(inputs["k"], np.float32)
    v = np.asarray(inputs["v"], np.float32)
    Wq = np.asarray(inputs["Wq"], np.float32).astype(_BF)
    Wk = _f8(np.asarray(inputs["Wk"], np.float32) * 64.0)
    Wv = _f8(np.asarray(inputs["Wv"], np.float32) * 64.0)
    Wfc = _f8(np.asarray(inputs["Wfc"], np.float32) * 64.0)
    fp = {n: np.asarray(inputs[n], np.float32)
          for n in ("bq", "bk", "bv", "gamma", "beta")}
    bfch = (np.asarray(inputs["bfc"], np.float32) * 4096.0).astype(_BF)
    ident = (np.eye(128, dtype=np.float32) * 4096.0).astype(_BF)

    in_maps = []
    for c in range(_NCORES):
        b, half = divmod(c, 2)
        t0 = half * _T
        in_maps.append({
            "qT": np.ascontiguousarray(q[b, t0:t0 + _T].T).astype(_BF),
            "kT": _f8(np.ascontiguousarray(k[b].T)),
            "vT": _f8(np.ascontiguousarray(v[b].T)),
            "Wq": Wq, "Wk": Wk, "Wv": Wv, "Wfc": Wfc,
            "bfch": bfch, "ident": ident, **fp,
        })

    res = run_bass_kernel_spmd(nc, in_maps, core_ids=list(range(_NCORES)),
                               trace=trace)
    out = np.empty((_B, _S, _D), np.float32)
    for c in range(_NCORES):
        b, half = divmod(c, 2)
        out[b, half * _T:(half + 1) * _T] = res.results[c]["out"]
    return out, res.exec_time_ns


def kernel(**inputs) -> np.ndarray:
    out, _ = _execute(inputs, trace=False)
    return out


# revision 17
# speedup vs baseline: 1.4684x; 1.0528x over previous
"""Trainium2 Bass kernel for nn_AttentionLayer (B=4, S=2048, D=1024, H=16).

Self-contained: builds and compiles an SPMD Bass/Tile program once, then
runs it across 8 NeuronCores via run_bass_kernel_spmd.

Sharding (no collectives): core c handles batch b = c // 2 and query-token
half c % 2 (1024 query tokens). Each core receives pre-transposed fp8
activations (x^T slices) plus fp8/bf16 weights, computes its [1024, 1024]
slice of the final layernorm output in fp32, and the host reassembles.

v3 pipeline:
- K/Q^T/V projections and the FC context matmuls run in fp8 (e4m3) with
  DoubleRow perf mode: operands are staged as [128, 2, N] tiles holding
  two 128-row contraction slabs, halving matmul streaming time. Weights
  are host-prescaled by 64 (fp8 range); evacuations rescale by 1/64.
- Attention per head pair: scores^T = Kh @ Qh^T in bf16 with both heads'
  K=64 matmuls packed into one PSUM tile per query chunk — shared WAR
  deps make the scheduler emit them adjacently, so they row-tile onto
  disjoint PE-array halves and run concurrently.
- exp head A on ScalarE (native, bf16 out); head B on VectorE via a
  Schraudolph bit trick (one tensor_scalar fp32->int16 whose bits are
  bf16 exp values; the ~3% sawtooth error cancels in softmax).
- attn@V in bf16 with a per-head ones column producing denominators.
- Softmax normalization deferred: denominators roundtrip DRAM (bf16),
  reciprocal via a magic-number bit trick fused with the x64 fp8 scale,
  producing normalized fp8 context tiles in DoubleRow pair layout.
- FC: fp8 DR ctx matmuls + residual via identity matmuls (transposing
  Q^T tiles, identity prescaled by 4096 to match the fp8 scales) + bfc
  via a K=1 ones matmul; layernorm absorbs the 4096 scale (eps * 4096^2).
"""

import numpy as np
import ml_dtypes


from contextlib import ExitStack

import concourse.bass as bass
import concourse.tile as tile
import concourse.mybir as mybir
from concourse import bacc

F32 = mybir.dt.float32
BF16 = mybir.dt.bfloat16
I16 = mybir.dt.int16
I8 = mybir.dt.int8
F8 = mybir.dt.float8e4
DR = mybir.MatmulPerfMode.DoubleRow
AF = mybir.ActivationFunctionType
ALU = mybir.AluOpType

LOG2E = 1.4426950408889634
# exp(x/8) ~= bf16_bits(int16(x * SCH_A + SCH_B)) (Schraudolph, bf16 top bits)
SCH_A = float((1 << 23) * LOG2E) * 0.125 / 65536.0
SCH_B = (float(127 << 23) - 366393.0) / 65536.0
# same trick to fp8e4m3 bits directly (int8 out): exp(x/8) ~= f8_bits(i8)
SCH_A8 = SCH_A / 16.0
SCH_B8 = (SCH_B - 15360.0) / 16.0
# 1/x ~= bf16_bits(RCP_MAGIC - bf16_bits(x)), x > 0 (max rel err ~5%)
RCP_MAGIC = 0x7EF3
WSCALE = 64.0          # host fp8 weight prescale
LAM = WSCALE * WSCALE  # fc psum scale (ctx*64 @ Wfc*64); LN is scale-invariant


def bcast_ap(ap: bass.AP, parts: int) -> bass.AP:
    """Partition-broadcast a [1, N]-shaped DRAM AP to [parts, N]."""
    return bass.AP(tensor=ap.tensor, offset=ap.offset,
                   ap=[[0, parts]] + list(ap.ap[-1:]))


def nsplits(total, cap=512):
    return [(i, min(cap, total - i)) for i in range(0, total, cap)]


def build(T=1024, S=2048, D=1024, H=16, DK=64, n_cores=8, eps=1e-5,
          trn_type="TRN2", apply_affine=True, apply_bfc=True):
    assert DK == 64 and H % 2 == 0 and D == H * DK
    G = D // 256      # DoubleRow contraction groups (256 rows each)
    EB = D // 128     # e blocks (projection output chunks); == H//2
    TB = T // 128
    SB = S // 128
    PAIRS = H // 2
    VW = 65           # per-head vp stripe: 64 v columns + 1 ones column

    nc = bacc.Bacc(trn_type, target_bir_lowering=False, debug=False,
                   num_devices=n_cores)

    qT = nc.dram_tensor("qT", [D, T], BF16, kind="ExternalInput").ap()
    kT = nc.dram_tensor("kT", [D, S], F8, kind="ExternalInput").ap()
    vT = nc.dram_tensor("vT", [D, S], F8, kind="ExternalInput").ap()
    Wq = nc.dram_tensor("Wq", [D, D], BF16, kind="ExternalInput").ap()
    Wk = nc.dram_tensor("Wk", [D, D], F8, kind="ExternalInput").ap()
    Wv = nc.dram_tensor("Wv", [D, D], F8, kind="ExternalInput").ap()
    Wfc = nc.dram_tensor("Wfc", [D, D], F8, kind="ExternalInput").ap()
    bq = nc.dram_tensor("bq", [D], F32, kind="ExternalInput").ap()
    bk = nc.dram_tensor("bk", [D], F32, kind="ExternalInput").ap()
    bv = nc.dram_tensor("bv", [D], F32, kind="ExternalInput").ap()
    bfch = nc.dram_tensor("bfch", [D], BF16, kind="ExternalInput").ap()
    gamma = nc.dram_tensor("gamma", [D], F32, kind="ExternalInput").ap()
    beta = nc.dram_tensor("beta", [D], F32, kind="ExternalInput").ap()
    ident = nc.dram_tensor("ident", [128, 128], BF16, kind="ExternalInput").ap()
    out = nc.dram_tensor("out", [T, D], F32, kind="ExternalOutput").ap()

    den_dram = nc.dram_tensor("den_scratch", [H, T], BF16).ap()

    def load_dr(pool, src, n, tagp, chunk=None):
        """Load fp8 [D, n] DRAM tensor into G [128, 2, n] DoubleRow tiles."""
        tiles = [pool.tile([128, 2, n], F8, tag=f"{tagp}{g}", name=f"{tagp}{g}")
                 for g in range(G)]
        for c0, cn in nsplits(n, chunk or n):
            for g in range(G):
                t = tiles[g]
                nc.sync.dma_start(out=t[:, 0, c0:c0 + cn],
                                  in_=src[g * 256:g * 256 + 128, c0:c0 + cn])
                nc.sync.dma_start(out=t[:, 1, c0:c0 + cn],
                                  in_=src[g * 256 + 128:g * 256 + 256,
                                          c0:c0 + cn])
        return tiles

    with tile.TileContext(nc) as tc, ExitStack() as ctx:
        pconst = ctx.enter_context(tc.tile_pool(name="const", bufs=1))
        ppers = ctx.enter_context(tc.tile_pool(name="persist", bufs=1))

        # ---- tiny constants -------------------------------------------
        bqT = pconst.tile([128, EB], F32, tag="bqT", name="bqT")
        nc.sync.dma_start(out=bqT, in_=bq.rearrange("(e p) -> p e", p=128))
        bkT = pconst.tile([128, EB], F32, tag="bkT", name="bkT")
        nc.sync.dma_start(out=bkT, in_=bk.rearrange("(e p) -> p e", p=128))
        eps_t = pconst.tile([128, 1], F32, tag="eps", name="eps")
        nc.vector.memset(eps_t, eps * LAM * LAM)
        i_sb = pconst.tile([128, 128], BF16, tag="ident", name="ident")
        nc.sync.dma_start(out=i_sb, in_=ident)
        ones1 = pconst.tile([1, 128], BF16, tag="ones1", name="ones1")
        nc.vector.memset(ones1, 1.0)
        bfc_sb = pconst.tile([1, D], BF16, tag="bfc_sb", name="bfc_sb")
        nc.sync.dma_start(out=bfc_sb, in_=bcast_ap(bfch, 1))

        # ---- persistent tiles -----------------------------------------
        kpT_sb = [ppers.tile([128, S], BF16, tag=f"kpT{e}", name=f"kpT{e}")
                  for e in range(EB)]
        vp_dr = [ppers.tile([128, 2, H * VW], F8, tag=f"vp{s2}",
                            name=f"vp{s2}") for s2 in range(SB // 2)]
        ctxT_sb = [ppers.tile([128, T], BF16, tag=f"ctxT{e}", name=f"ctxT{e}")
                   for e in range(EB)]
        ctx8_sb = [ppers.tile([128, 2, T], F8, tag=f"ctx8_{g}",
                              name=f"ctx8_{g}") for g in range(G)]
        qpT_sb = [ppers.tile([128, T], BF16, tag=f"qpT{j}", name=f"qpT{j}")
                  for j in range(PAIRS)]

        # ================= K projection (e-outer, fp8 DR) ==============
        with tc.tile_pool(name="wk", bufs=1) as pw, \
             tc.tile_pool(name="kx", bufs=1) as pkx, \
             tc.tile_pool(name="qx", bufs=1) as pqx, \
             tc.tile_pool(name="wq", bufs=1) as pwq, \
             tc.tile_pool(name="kps", bufs=2, space="PSUM") as pps, \
             tc.tile_pool(name="qps", bufs=2, space="PSUM") as pqps:
            wk_dr = load_dr(pw, Wk, D, "wk")
            kx_dr = load_dr(pkx, kT, S, "kx", chunk=1024)
            DB = D // 128
            qx_sb = [pqx.tile([128, T], BF16, tag=f"qx{d}", name=f"qx{d}")
                     for d in range(DB)]
            wq_sb = [pwq.tile([128, D], BF16, tag=f"wq{d}", name=f"wq{d}")
                     for d in range(DB)]
            for d in range(DB):
                nc.sync.dma_start(out=qx_sb[d], in_=qT[d * 128:(d + 1) * 128, :])
                nc.sync.dma_start(out=wq_sb[d], in_=Wq[d * 128:(d + 1) * 128, :])
            CK = min(S, 1024)
            for e in range(EB):
                ecol = slice(e * 128, (e + 1) * 128)
                for c0, cn in nsplits(S, CK):
                    ps = pps.tile([128, CK], F32, tag="kpT_ps", name="kpT_ps")
                    for g in range(G):
                        for n0, nn in nsplits(cn):
                            nc.tensor.matmul(
                                ps[:, n0:n0 + nn],
                                lhsT=wk_dr[g][:, :, ecol],
                                rhs=kx_dr[g][:, :, c0 + n0:c0 + n0 + nn],
                                start=(g == 0), stop=(g == G - 1),
                                perf_mode=DR)
                    nc.scalar.activation(
                        out=kpT_sb[e][:, c0:c0 + cn], in_=ps[:, 0:cn],
                        func=AF.Identity, scale=1.0 / WSCALE,
                        bias=bkT[:, e:e + 1])

            # ============= Q^T projection (all pairs, fp8 DR) ==========
            for j in range(PAIRS):
                qps = pqps.tile([128, T], F32, tag="qps", name="qps")
                for d in range(DB):
                    for n0, nn in nsplits(T):
                        nc.tensor.matmul(
                            qps[:, n0:n0 + nn],
                            lhsT=wq_sb[d][:, j * 128:(j + 1) * 128],
                            rhs=qx_sb[d][:, n0:n0 + nn],
                            start=(d == 0), stop=(d == DB - 1))
                nc.scalar.activation(out=qpT_sb[j], in_=qps,
                                     func=AF.Identity, scale=1.0,
                                     bias=bqT[:, j:j + 1])

        # ================= V projection (natural layout, fp8 DR) ========
        with tc.tile_pool(name="wv", bufs=1) as pw, \
             tc.tile_pool(name="vx", bufs=1) as pvx, \
             tc.tile_pool(name="vbc", bufs=1) as pvbc, \
             tc.tile_pool(name="vps", bufs=3, space="PSUM") as pps:
            bv_bc = pvbc.tile([128, D], F32, tag="bv_bc", name="bv_bc")
            nc.gpsimd.dma_start(out=bv_bc, in_=bcast_ap(bv, 128))
            wv_dr = load_dr(pw, Wv, D, "wv")
            vx_dr = load_dr(pvx, vT, S, "vx")
            for s in range(SB):
                ps = pps.tile([128, D], F32, tag="vp_ps", name="vp_ps")
                for g in range(G):
                    for n0, nn in nsplits(D):
                        nc.tensor.matmul(
                            ps[:, n0:n0 + nn],
                            lhsT=vx_dr[g][:, :, s * 128:(s + 1) * 128],
                            rhs=wv_dr[g][:, :, n0:n0 + nn],
                            start=(g == 0), stop=(g == G - 1), perf_mode=DR)
                vr = vp_dr[s // 2].rearrange("p k (h c) -> p k h c",
                                             c=VW)
                nc.vector.scalar_tensor_tensor(
                    out=vr[:, s % 2, :, 0:64],
                    in0=ps.rearrange("p (h c) -> p h c", c=DK),
                    scalar=1.0 / WSCALE,
                    in1=bv_bc.rearrange("p (h c) -> p h c", c=DK),
                    op0=ALU.mult, op1=ALU.add)
                nc.vector.memset(vr[:, s % 2, :, 64:65], 1.0)

        # ================= attention ====================================
        pwfc = ctx.enter_context(tc.tile_pool(name="wfc", bufs=1))
        wfc_dr = []
        for g in range(G):
            t = pwfc.tile([128, 2, D], F8, tag=f"wfc{g}", name=f"wfc{g}")
            nc.gpsimd.dma_start(out=t[:, 0, :],
                                in_=Wfc[g * 256:g * 256 + 128, :])
            nc.gpsimd.dma_start(out=t[:, 1, :],
                                in_=Wfc[g * 256 + 128:g * 256 + 256, :])
            wfc_dr.append(t)

        with tc.tile_pool(name="scp", bufs=3, space="PSUM") as psc, \
             tc.tile_pool(name="cxps", bufs=1, space="PSUM") as pcx, \
             tc.tile_pool(name="atA", bufs=2) as pata, \
             tc.tile_pool(name="norm", bufs=2) as pnm, \
             tc.tile_pool(name="ctmp", bufs=2) as ptmp:
            HT = T // 2
            for j in range(PAIRS):
                kA = kpT_sb[j][0:64, :]
                kB = kpT_sb[j][64:128, :]
                for th in range(2):
                    tsl = slice(th * HT, (th + 1) * HT)
                    qA = qpT_sb[j][0:64, tsl]
                    qB = qpT_sb[j][64:128, tsl]
                    cxa = pcx.tile([VW, HT], F32, tag="cxA", name="cxA")
                    cxb = pcx.tile([VW, HT], F32, tag="cxB", name="cxB")
                    at_q = []

                    def attnv(m, cxa=cxa, cxb=cxb, j=j, at_q=at_q):
                        at8 = at_q[m]
                        vrA = vp_dr[m][:, :, 2 * j * VW:2 * j * VW + VW]
                        vrB = vp_dr[m][:, :,
                                       (2 * j + 1) * VW:(2 * j + 2) * VW]
                        st, sp = (m == 0), (m == SB // 2 - 1)
                        nc.tensor.matmul(cxa, lhsT=vrA, rhs=at8[:, :, 0:HT],
                                         start=st, stop=sp, perf_mode=DR)
                        nc.tensor.matmul(cxb, lhsT=vrB, rhs=at8[:, :, HT:T],
                                         start=st, stop=sp, perf_mode=DR)

                    for kb in range(SB):
                        kblk = slice(kb * 128, (kb + 1) * 128)
                        # one score tile holds both heads' chunk (A then
                        # B): shared WAR deps keep the two K=64 matmuls
                        # adjacent, so they row-tile concurrently; the
                        # 3-deep pool breaks the scores->exp->scores
                        # serial chain.
                        sc = psc.tile([128, T], F32, tag="sc", name="sc")
                        nc.tensor.matmul(sc[:, 0:HT], lhsT=kA[:, kblk],
                                         rhs=qA, start=True, stop=True)
                        nc.tensor.matmul(sc[:, HT:T], lhsT=kB[:, kblk],
                                         rhs=qB, start=True, stop=True)
                        # exp of the whole tile in ONE call, alternating
                        # engines per kb: ScalarE native exp (fp8 out) on
                        # even kb, VectorE Schraudolph-to-fp8-bits on odd.
                        sl = kb % 2
                        if sl == 0:
                            at8 = pata.tile([128, 2, T], F8, tag="at8",
                                            name="at8")
                            at_q.append(at8)
                            nc.scalar.activation(out=at8[:, 0, :], in_=sc,
                                                 func=AF.Exp, scale=0.125)
                        else:
                            nc.vector.tensor_scalar(
                                out=at8.bitcast(I8)[:, 1, :], in0=sc,
                                scalar1=SCH_A8, scalar2=SCH_B8,
                                op0=ALU.mult, op1=ALU.add)
                        # attn@V (fp8 DoubleRow, 256-key contraction),
                        # lagged one kb-pair so its inputs are complete
                        # and it never stalls TensorE's in-order queue.
                        if sl == 1 and kb // 2 >= 1:
                            attnv(kb // 2 - 1)
                    attnv(SB // 2 - 1)
                    # evacuate ctx + denominators: head A via DVE, head B
                    # via ScalarE (the PSUM-capable engines); den rows
                    # ride along in the [65, HT] staging copies.
                    stga = ptmp.tile([VW, HT], BF16, tag="stga", name="stga")
                    nc.vector.tensor_copy(out=stga, in_=cxa)
                    stgb = ptmp.tile([VW, HT], BF16, tag="stgb", name="stgb")
                    nc.scalar.activation(out=stgb, in_=cxb, func=AF.Copy)
                    nc.sync.dma_start(out=ctxT_sb[j][0:64, tsl],
                                      in_=stga[0:64, :])
                    nc.sync.dma_start(out=ctxT_sb[j][64:128, tsl],
                                      in_=stgb[0:64, :])
                    nc.gpsimd.dma_start(out=den_dram[2 * j, tsl],
                                        in_=stga[64:65, :])
                    nc.gpsimd.dma_start(out=den_dram[2 * j + 1, tsl],
                                        in_=stgb[64:65, :])
                    # deferred softmax normalization: magic-number bf16
                    # reciprocal of broadcast denominators, the x64 fp8
                    # ctx scale folded into the magic constant.
                    dbc = pnm.tile([128, HT], BF16, tag="dbc", name="dbc")
                    nc.gpsimd.dma_start(
                        out=dbc[0:64, :],
                        in_=bcast_ap(den_dram[2 * j:2 * j + 1, tsl], 64))
                    nc.gpsimd.dma_start(
                        out=dbc[64:128, :],
                        in_=bcast_ap(den_dram[2 * j + 1:2 * j + 2, tsl], 64))
                    rbc = pnm.tile([128, HT], I16, tag="rbc", name="rbc")
                    nc.gpsimd.tensor_scalar(out=rbc, in0=dbc.bitcast(I16),
                                            scalar1=-1,
                                            scalar2=RCP_MAGIC + (6 << 7),
                                            op0=ALU.mult, op1=ALU.add)
                    nc.gpsimd.tensor_mul(out=ctx8_sb[j // 2][:, j % 2, tsl],
                                         in0=ctxT_sb[j][:, tsl],
                                         in1=rbc.bitcast(BF16))

        # ================= FC + residual + layernorm ====================
        with tc.tile_pool(name="fcps", bufs=2, space="PSUM") as pfc, \
             tc.tile_pool(name="lnbc", bufs=1) as plnb, \
             tc.tile_pool(name="xln", bufs=2) as px, \
             tc.tile_pool(name="stat", bufs=4) as pst:
            if apply_affine:
                gamma_bc = plnb.tile([128, D], F32, tag="gamma_bc",
                                     name="gamma_bc")
                nc.gpsimd.dma_start(out=gamma_bc, in_=bcast_ap(gamma, 128))
                beta_bc = plnb.tile([128, D], F32, tag="beta_bc",
                                    name="beta_bc")
                nc.gpsimd.dma_start(out=beta_bc, in_=bcast_ap(beta, 128))

            for t in range(TB):
                tblk = slice(t * 128, (t + 1) * 128)
                fc = pfc.tile([128, D], F32, tag="fc", name="fc")
                for c0, cn in nsplits(D):
                    for g in range(G):
                        nc.tensor.matmul(
                            fc[:, c0:c0 + cn],
                            lhsT=ctx8_sb[g][:, :, tblk],
                            rhs=wfc_dr[g][:, :, c0:c0 + cn],
                            start=(g == 0), stop=False, perf_mode=DR)
                    # residual: transpose qpT pair blocks via identity
                    # (identity prescaled by LAM to match fp8 scales)
                    for jj in range(c0 // 128, (c0 + cn) // 128):
                        nc.tensor.matmul(
                            fc[:, jj * 128:(jj + 1) * 128],
                            lhsT=qpT_sb[jj][:, tblk], rhs=i_sb,
                            start=False, stop=False)
                    if apply_bfc:
                        # bfc bias via K=1 ones matmul (marks group end)
                        nc.tensor.matmul(
                            fc[:, c0:c0 + cn], lhsT=ones1,
                            rhs=bfc_sb[0:1, c0:c0 + cn], start=False,
                            stop=True)
                ngr = max(D // 512, 1)
                gsz = min(D, 512)
                stats = pst.tile([128, ngr, 6], F32, tag="stats", name="stats")
                for g in range(ngr):
                    nc.vector.bn_stats(out=stats[:, g, :],
                                       in_=fc[:, g * gsz:(g + 1) * gsz])
                mv = pst.tile([128, 2], F32, tag="mv", name="mv")
                nc.vector.bn_aggr(out=mv, in_=stats)
                rstd = pst.tile([128, 1], F32, tag="rstd", name="rstd")
                nc.scalar.activation(out=rstd, in_=mv[:, 1:2], func=AF.Sqrt,
                                     bias=eps_t, scale=1.0)
                nc.vector.reciprocal(out=rstd, in_=rstd)
                nmr = pst.tile([128, 1], F32, tag="nmr", name="nmr")
                nc.vector.tensor_scalar(out=nmr, in0=mv[:, 0:1],
                                        scalar1=rstd, scalar2=-1.0,
                                        op0=ALU.mult, op1=ALU.mult)
                xn = px.tile([128, D], F32, tag="xn", name="xn")
                nc.scalar.activation(out=xn, in_=fc, func=AF.Identity,
                                     scale=rstd, bias=nmr)
                if apply_affine:
                    xg = px.tile([128, D], F32, tag="xg", name="xg")
                    nc.vector.tensor_mul(out=xg, in0=xn, in1=gamma_bc)
                    nc.gpsimd.tensor_add(out=xg, in0=xg, in1=beta_bc)
                else:
                    xg = xn
                nc.sync.dma_start(out=out[tblk, :], in_=xg)

    nc.compile()
    return nc


_B, _S, _D, _H, _DK = 4, 2048, 1024, 16, 64
_T = _S // 2
_NCORES = 8
_BF = ml_dtypes.bfloat16
_F8 = ml_dtypes.float8_e4m3

_nc_cache = {}


def _get_nc(apply_affine, apply_bfc):
    key = (apply_affine, apply_bfc)
    if key not in _nc_cache:
        _nc_cache[key] = build(T=_T, S=_S, D=_D, H=_H, DK=_DK,
                               n_cores=_NCORES, apply_affine=apply_affine,
                               apply_bfc=apply_bfc)
    return _nc_cache[key]


def _f8(x):
    return np.clip(x, -240.0, 240.0).astype(_F8)


def _execute(inputs, trace=False):
    from concourse.bass_utils import run_bass_kernel_spmd

    gamma_h = np.asarray(inputs["gamma"], np.float32)
    beta_h = np.asarray(inputs["beta"], np.float32)
    aff = not (np.all(gamma_h == 1.0) and np.all(beta_h == 0.0))
    bfc_h = np.asarray(inputs["bfc"], np.float32)
    nc = _get_nc(aff, bool(np.any(bfc_h != 0.0)))
    q = np.asarray(inputs["q"], np.float32)
    k = np.asarray(inputs["k"], np.float32)
    v = np.asarray(inputs["v"], np.float32)
    Wq = np.asarray(inputs["Wq"], np.float32).astype(_BF)
    Wk = _f8(np.asarray(inputs["Wk"], np.float32) * 64.0)
    Wv = _f8(np.asarray(inputs["Wv"], np.float32) * 64.0)
    Wfc = _f8(np.asarray(inputs["Wfc"], np.float32) * 64.0)
    fp = {n: np.asarray(inputs[n], np.float32)
          for n in ("bq", "bk", "bv", "gamma", "beta")}
    bfch = (np.asarray(inputs["bfc"], np.float32) * 4096.0).astype(_BF)
    ident = (np.eye(128, dtype=np.float32) * 4096.0).astype(_BF)

    in_maps = []
    for c in range(_NCORES):
        b, half = divmod(c, 2)
        t0 = half * _T
        in_maps.append({
            "qT": np.ascontiguousarray(q[b, t0:t0 + _T].T).astype(_BF),
            "kT": _f8(np.ascontiguousarray(k[b].T)),
            "vT": _f8(np.ascontiguousarray(v[b].T)),
            "Wq": Wq, "Wk": Wk, "Wv": Wv, "Wfc": Wfc,
            "bfch": bfch, "ident": ident, **fp,
        })

    res = run_bass_kernel_spmd(nc, in_maps, core_ids=list(range(_NCORES)),
                               trace=trace)
    out = np.empty((_B, _S, _D), np.float32)
    for c in range(_NCORES):
        b, half = divmod(c, 2)
        out[b, half * _T:(half + 1) * _T] = res.results[c]["out"]
    return out, res.exec_time_ns


def kernel(**inputs) -> np.ndarray:
    out, _ = _execute(inputs, trace=False)
    return out
